# revision 26
# baseline (speedup 1.0000x reference)
"""Trainium2 Bass kernel for a full MHA block (QKV proj + softmax attention +
output proj + residual + LayerNorm), B=2, S=4096, E=512, H=8, D=64.

Sharding: sequence-parallel over 8 cores (4 seq shards x 2 batches). Each core
owns R=1024 query rows of one batch, recomputes K/V for the full context
(avoids all cross-core communication), and writes its own [R, E] output slice.

Schedule (the point of this version): the ScalarE exp stream (256 x ~1us) and
the PE matmul stream (~270us at full clock) are the two near-equal critical
resources. All projection work (K/Q/V/O) is diced into small "filler" units
and pumped just-in-time between attention groups so the PE queue never drains
(keeping the PE at its ramped 2.4GHz clock) while ScalarE does back-to-back
exps. Startup is minimal (first scores ~6us in); the O-proj/LayerNorm output
path streams during the last head-pair so the tail is short.

Per-core layout (as the previous version):
  - x^T via XBAR DMA-transpose (bf16) -> [128, 4, S]; all DMA issue kept off
    the Scalar queue so ScalarE only runs Exp.
  - K^T/Q^T head-pair-major [128(=2 heads x 64), t]; scores transposed
    S_T[t, s] = kT.T @ qT with both heads resident in disjoint PE quadrants
    (tile_position), PSUM out [128, 2, 512] per head.
  - exp on ScalarE straight from PSUM (scale=1/8, bias=-2 folded in), fp8 out.
  - A@V: DoubleRow fp8, lhsT = [V_h | ones] so ctx row 64 accumulates the
    softmax denominator for free; accumulate whole s-block in PSUM.
  - normalize: DVE reciprocal of the denominator row [1, sblk] read straight
    from PSUM, partition-broadcast via one SBUF->SBUF DMA, DVE multiply ->
    ctx^T bf16 (also drains the PSUM accumulator).
  - O-proj per-head K=64 matmuls + residual + LayerNorm; 1/sigma via DVE
    fast-inverse-sqrt (2 Newton steps) so ScalarE never loads the Sqrt table.
"""

import sys

sys.path.insert(0, "/opt/trn_rl_repo")

from collections import deque

import numpy as np
import ml_dtypes

import concourse.bass as bass
import concourse.bacc as bacc
import concourse.mybir as mybir
import concourse.tile as tile
from concourse.bass import ds, ts

# Problem constants (hardcoded per harness contract)
B = 2
S = 4096
E = 512
H = 8
D = 64
N_CORES = 8
SEQ_SHARDS = N_CORES // B
R = S // SEQ_SHARDS  # 1024 own query rows per core

F32 = mybir.dt.float32
BF16 = mybir.dt.bfloat16
FP8 = mybir.dt.float8e4
U32 = mybir.dt.uint32
VP = 80  # padded V columns (64 V + 1 ones + pad to a 16-multiple for DoubleRow)
# exp(s/8 + SHIFT): keeps exp outputs well under the fp8e4 max (448); the
# shift cancels in softmax. -2.0 left only 7% margin on the real data (max
# score 64.25 on batch1 -> exp=416) and HW exp/rounding pushed one value to
# NaN; -3.0 gives ~4x margin at no accuracy cost (fp8 precision is relative).
EXP_SHIFT = -3.0
RSQRT_MAGIC = 0x5F3759DF
AF = mybir.ActivationFunctionType
ALU = mybir.AluOpType


def build_mha(nc, seq=S, rows=R, exp_group=2, sblk=512):
    """Emit the Tile program. seq/rows shrinkable for simulation."""
    assert exp_group == 2, "schedule is built around t-tile pairs"
    P = 128
    EC = E // P           # 4 e_in chunks
    HPAIRS = H // 2       # 4 head-pair blocks (=e_out blocks of 128)
    TT = seq // P         # t tiles
    NPAIR = TT // 2       # t-tile pairs (DoubleRow granularity)
    kblk = min(512, seq)
    TB = seq // kblk      # t blocks for K-proj
    qblk = min(512, rows)
    QB = rows // qblk     # r blocks for Q-proj
    sblk = min(sblk, rows)
    SB = rows // sblk     # s blocks per core
    ST = rows // P        # s tiles for O-proj/LN
    scale = 1.0 / np.sqrt(D)

    # ---- DRAM I/O ----
    x_bf = nc.dram_tensor("x_bf", [seq, E], BF16, kind="ExternalInput").ap()
    xo_bf = nc.dram_tensor("xo_bf", [rows, E], BF16, kind="ExternalInput").ap()
    xo_f32 = nc.dram_tensor("xo_f32", [rows, E], F32, kind="ExternalInput").ap()
    wq = nc.dram_tensor("wq_bf", [E, E], BF16, kind="ExternalInput").ap()
    wk = nc.dram_tensor("wk_bf", [E, E], BF16, kind="ExternalInput").ap()
    wv = nc.dram_tensor("wv_bf", [E, E], BF16, kind="ExternalInput").ap()
    wo = nc.dram_tensor("wo_bf", [E, E], BF16, kind="ExternalInput").ap()
    bq = nc.dram_tensor("bq", [E], F32, kind="ExternalInput").ap()
    bk = nc.dram_tensor("bk", [E], F32, kind="ExternalInput").ap()
    bv = nc.dram_tensor("bv", [E], F32, kind="ExternalInput").ap()
    bo = nc.dram_tensor("bo", [E], F32, kind="ExternalInput").ap()
    ln_g = nc.dram_tensor("ln_g", [E], F32, kind="ExternalInput").ap()
    ln_b = nc.dram_tensor("ln_b", [E], F32, kind="ExternalInput").ap()
    y_out = nc.dram_tensor("y", [rows, E], F32, kind="ExternalOutput").ap()

    with tile.TileContext(nc) as tc:
        with (
            tc.tile_pool(name="singles", bufs=1) as singles,
            tc.tile_pool(name="kqv", bufs=1) as kqv,
            tc.tile_pool(name="vtiles", bufs=NPAIR) as vtiles,
            tc.tile_pool(name="at", bufs=6) as atp,
            tc.tile_pool(name="norm", bufs=4) as normp,
            tc.tile_pool(name="ctx", bufs=4) as ctxp,
            tc.tile_pool(name="yout", bufs=3) as youtp,
            tc.tile_pool(name="dram", bufs=4, space="DRAM") as dramp,
            tc.tile_pool(name="stg", bufs=2, space="PSUM") as stg,
            tc.tile_pool(name="acc", bufs=2, space="PSUM") as accp,
            tc.tile_pool(name="util", bufs=2, space="PSUM") as util,
        ):
            # ---------- weights / constants ----------
            # The gpsimd SWDGE queue drains one DMA at a time (~several us
            # each), so order it critical-first: tiny biases, then the
            # attention-path weights, then everything the output stage needs.
            bk_sb = singles.tile([P, EC], F32, name="bk_sb")
            bq_sb = singles.tile([P, EC], F32, name="bq_sb")
            nc.gpsimd.dma_start(bk_sb, bk.rearrange("(c p) -> p c", p=P))
            nc.gpsimd.dma_start(bq_sb, bq.rearrange("(c p) -> p c", p=P))
            wk_sb = singles.tile([P, EC, E], BF16, name="wk_sb")
            wq_sb = singles.tile([P, EC, E], BF16, name="wq_sb")
            wv_sb = singles.tile([P, EC, E], BF16, name="wv_sb")
            nc.gpsimd.dma_start(wk_sb, wk.rearrange("(c p) e -> p c e", p=P))
            nc.gpsimd.dma_start(wq_sb, wq.rearrange("(c p) e -> p c e", p=P))
            nc.gpsimd.dma_start(wv_sb, wv.rearrange("(c p) e -> p c e", p=P))
            bv_bc = singles.tile([P, E], F32, name="bv_bc")
            nc.gpsimd.dma_start(out=bv_bc, in_=bv[None, :].to_broadcast((P, E)))

            # x^T transposes: only the two HWDGE queues (SP/Activation) can
            # issue XBAR transposes, and each stream moves ~one 512-row chunk
            # per ~2.7us, so split chunks across both. Issue-gen is cheap and
            # the Activation queue is idle until the first scores anyway.
            xT = singles.tile([P, EC, seq], BF16, name="xT")
            xoT = singles.tile([P, EC, rows], BF16, name="xoT")
            for c in range(EC):
                nc.scalar.dma_start_transpose(
                    xT[:, c, 0 : min(512, seq)], x_bf[0 : min(512, seq), ds(c * P, P)]
                )
            for c in range(EC):
                nc.sync.dma_start_transpose(xoT[:, c, :], xo_bf[:, ds(c * P, P)])
            lo = min(512, seq)
            queues = [nc.scalar, nc.sync]
            qi = 1
            while lo < seq:
                hi = min(lo + 512, seq)
                for c in range(EC):
                    queues[qi].dma_start_transpose(
                        xT[:, c, ds(lo, hi - lo)],
                        x_bf[ds(lo, hi - lo), ds(c * P, P)],
                    )
                qi ^= 1
                lo = hi

            # wo in per-head layout: [64, H, E]
            wo_sb = singles.tile([D, H, E], BF16, name="wo_sb")
            nc.gpsimd.dma_start(wo_sb, wo.rearrange("(h p) e -> p h e", p=D))
            # free-dim broadcast tiles for the output stage
            bo_bc = singles.tile([P, E], F32, name="bo_bc")
            g_bc = singles.tile([P, E], F32, name="g_bc")
            b_bc = singles.tile([P, E], F32, name="b_bc")
            for dst, src in ((bo_bc, bo), (g_bc, ln_g), (b_bc, ln_b)):
                nc.gpsimd.dma_start(out=dst, in_=src[None, :].to_broadcast((P, E)))
            # residual rows prefetch (f32) for the O-proj stage, with the
            # output bias pre-folded in (DVE is idle during startup)
            xo_res = singles.tile([P, ST, E], F32, name="xo_res")
            for st in range(ST):
                nc.gpsimd.dma_start(xo_res[:, st, :], xo_f32[ts(st, P), :])
                nc.vector.tensor_add(
                    out=xo_res[:, st, :], in0=xo_res[:, st, :], in1=bo_bc
                )
            shift_t = singles.tile([P, 1], F32, name="shift_t")
            nc.vector.memset(shift_t, EXP_SHIFT)
            eps_t = singles.tile([P, 1], F32, name="eps_t")
            nc.vector.memset(eps_t, 1e-5)

            # ---------- projection / output units (filler work) ----------
            kT = [kqv.tile([P, seq], BF16, name=f"kT_{hp}") for hp in range(HPAIRS)]
            qT = [kqv.tile([P, rows], BF16, name=f"qT_{hp}") for hp in range(HPAIRS)]
            ctxT = [kqv.tile([D, rows], BF16, name=f"ctxT_{h}") for h in range(H)]
            v_tiles = {}

            def emit_k(hp, tb):
                ps = util.tile([P, 512], F32, name="k_ps", tag="u")
                for c in range(EC):
                    nc.tensor.matmul(
                        ps[:, :kblk], lhsT=wk_sb[:, c, ds(hp * P, P)],
                        rhs=xT[:, c, ds(tb * kblk, kblk)],
                        start=(c == 0), stop=(c == EC - 1),
                    )
                nc.vector.tensor_tensor(
                    kT[hp][:, ds(tb * kblk, kblk)], ps[:, :kblk],
                    bk_sb[:, hp : hp + 1].to_broadcast((P, kblk)),
                    ALU.add,
                )

            def emit_q(hp, rb):
                ps = util.tile([P, 512], F32, name="q_ps", tag="u")
                for c in range(EC):
                    nc.tensor.matmul(
                        ps[:, :qblk], lhsT=wq_sb[:, c, ds(hp * P, P)],
                        rhs=xoT[:, c, ds(rb * qblk, qblk)],
                        start=(c == 0), stop=(c == EC - 1),
                    )
                nc.vector.tensor_tensor(
                    qT[hp][:, ds(rb * qblk, qblk)], ps[:, :qblk],
                    bq_sb[:, hp : hp + 1].to_broadcast((P, qblk)),
                    ALU.add,
                )

            def emit_v(t, hp):
                # V projection for 2 heads of one t-tile; fp8 DoubleRow pair
                # layout: v2[pair][p, i, h, 0:64] = v[t=2*pair+i], [..., 64]=1.
                pair, i = divmod(t, 2)
                if pair not in v_tiles:
                    vt = vtiles.tile([P, 2, H, VP], FP8, name=f"v_{pair}", tag="v")
                    # pad columns zeroed (junk would only land in unread PSUM
                    # rows, but keep the sim's uninitialized-read checks quiet)
                    nc.vector.memset(vt[:, :, :, D:VP], 0.0)
                    nc.vector.memset(vt[:, :, :, D : D + 1], 1.0)
                    v_tiles[pair] = vt
                vt = v_tiles[pair]
                ps = util.tile([P, 512], F32, name="v_ps", tag="u")
                for c in range(EC):
                    nc.tensor.matmul(
                        ps[:, 0 : 2 * D], lhsT=xT[:, c, ts(t, P)],
                        rhs=wv_sb[:, c, ds(hp * 2 * D, 2 * D)],
                        start=(c == 0), stop=(c == EC - 1),
                    )
                nc.vector.tensor_add(
                    out=vt[:, i, ds(hp * 2, 2), 0:D],
                    in0=ps[:, 0 : 2 * D].rearrange("p (h d) -> p h d", h=2),
                    in1=bv_bc[:, ds(hp * 2 * D, 2 * D)].rearrange(
                        "p (h d) -> p h d", h=2
                    ),
                )

            def emit_out(st):
                # O-proj + residual + LayerNorm for one 128-row output tile.
                ps = util.tile([P, E], F32, name="o_ps", tag="u")
                for h in range(H):
                    nc.tensor.matmul(
                        ps, lhsT=ctxT[h][:, ts(st, P)], rhs=wo_sb[:, h, :],
                        start=(h == 0), stop=(h == H - 1),
                    )
                y_t = youtp.tile([P, E], F32, name="y_t")
                nc.vector.tensor_add(out=y_t, in0=ps, in1=xo_res[:, st, :])
                stats = normp.tile([P, 6], F32, name="stats")
                nc.vector.bn_stats(out=stats, in_=y_t)
                mv = normp.tile([P, 2], F32, name="mv")
                nc.vector.bn_aggr(out=mv, in_=stats)
                std = normp.tile([P, 1], F32, name="std")
                nc.scalar.activation(
                    out=std, in_=mv[:, 1:2], func=AF.Sqrt, bias=eps_t
                )
                nc.vector.reciprocal(out=std, in_=std)
                nc.vector.tensor_tensor(
                    y_t, y_t, mv[:, 0:1].to_broadcast((P, E)), ALU.subtract
                )
                nc.vector.tensor_tensor(
                    y_t, y_t, std.to_broadcast((P, E)), ALU.mult
                )
                nc.vector.tensor_mul(out=y_t, in0=y_t, in1=g_bc)
                nc.vector.tensor_add(out=y_t, in0=y_t, in1=b_bc)
                nc.sync.dma_start(y_out[ts(st, P), :], y_t)

            # ---------- filler scheduling ----------
            # Emission order defines dependencies, so every filler carries a
            # hard deadline (hp, sb, group) by which it MUST have been
            # emitted; `pump` additionally drains fillers early to keep the
            # PE queue full during ScalarE-bound stretches.
            fillers = deque()  # (deadline, fn), kept deadline-sorted

            def run_due(key):
                while fillers and fillers[0][0] <= key:
                    fillers.popleft()[1]()

            # ---------- attention ----------
            def attention(hp, sb, pump, last_block=False):
                run_due((hp, sb, 0))
                ctx_ps = [
                    accp.tile([VP, sblk], F32, name=f"ctx_{h}", tag="ctx")
                    for h in range(2)
                ]
                pending = []  # (at_pair, pair_idx) awaiting A@V, ~2 behind

                def flush_av(last):
                    at_p, pr = pending.pop(0)
                    for h in range(2):
                        nc.tensor.matmul(
                            ctx_ps[h][:, :sblk],
                            lhsT=v_tiles[pr][:, :, hp * 2 + h, :],
                            rhs=at_p[h][:, :, :sblk],
                            start=(pr == 0), stop=last,
                            perf_mode=mybir.MatmulPerfMode.DoubleRow,
                        )

                for g in range(NPAIR):
                    run_due((hp, sb, g))
                    # independent work (fillers, the lagged A@V) goes BEFORE
                    # this group's scores: when the scores stall on the exp
                    # double-buffer, the PE queue head has nothing else left
                    # to wait behind, so the stall (and the DVFS downclock it
                    # causes) is as short as possible.
                    for _ in range(pump):
                        if fillers:
                            fillers.popleft()[1]()
                    if len(pending) > 2:
                        flush_av(False)
                    at_pair = []
                    for h in range(2):
                        st_t = stg.tile([P, 2, 512], F32, name=f"stg_{h}", tag="stg")
                        for j in range(2):
                            nc.tensor.matmul(
                                st_t[:, j, :sblk],
                                lhsT=kT[hp][ds(h * D, D), ts(2 * g + j, P)],
                                rhs=qT[hp][ds(h * D, D), ds(sb * sblk, sblk)],
                                start=True, stop=True,
                                tile_position=(h * D, 0),
                            )
                        at_t = atp.tile([P, 2, 512], FP8, name=f"at_{h}", tag="at")
                        nc.scalar.activation(
                            out=at_t[:, :, :sblk], in_=st_t[:, :, :sblk],
                            func=AF.Exp, scale=scale, bias=shift_t,
                        )
                        at_pair.append(at_t)
                    pending.append((at_pair, g))
                while pending:
                    flush_av(len(pending) == 1)
                # drain accumulators to SBUF immediately (frees PSUM banks),
                # then normalize: reciprocal of the denom row on all DVE
                # lanes via a DRAM bounce reshape, partition-broadcast back.
                ctx_sb = [ctxp.tile([D + 1, sblk], F32, name=f"cs_{h}", tag="cs")
                          for h in range(2)]
                for h in range(2):
                    nc.vector.tensor_copy(ctx_sb[h], ctx_ps[h][: D + 1, :sblk])
                for h in range(2):
                    # on the last block the Scalar queue is idle (no exps
                    # left), so run the two heads' bounce chains in parallel
                    # on separate HWDGE queues to shorten the tail
                    qeng = nc.scalar if (last_block and h == 1) else nc.sync
                    fw = sblk // P if sblk >= P else 1
                    pw = min(P, sblk)
                    dr = dramp.tile([sblk], F32, name="dr", tag="dr")
                    qeng.dma_start(out=dr[None, :], in_=ctx_sb[h][D : D + 1, :])
                    dn4 = normp.tile([P, fw], F32, name="dn4")
                    qeng.dma_start(
                        out=dn4[:pw, :], in_=dr.rearrange("(p f) -> p f", p=pw)
                    )
                    nc.vector.reciprocal(out=dn4[:pw, :], in_=dn4[:pw, :])
                    dr2 = dramp.tile([sblk], F32, name="dr2", tag="dr2")
                    qeng.dma_start(
                        out=dr2.rearrange("(p f) -> p f", p=pw), in_=dn4[:pw, :]
                    )
                    rb_t = normp.tile([D, sblk], F32, name="rb", tag="rb")
                    nc.gpsimd.dma_start(
                        out=rb_t, in_=dr2[None, :].to_broadcast((D, sblk))
                    )
                    nc.vector.tensor_mul(
                        out=ctxT[hp * 2 + h][:, ds(sb * sblk, sblk)],
                        in0=ctx_sb[h][0:D, :], in1=rb_t,
                    )

            # ---------- emission order ----------
            # Phase 1: the minimum for the first scores of (hp0, sb0).
            emit_k(0, 0)
            for rb in range(QB):
                emit_q(0, rb)
            emit_v(0, 0)
            emit_v(1, 0)

            emitted_out = set()
            tpb = kblk // P  # t-tiles per K block

            # Block (0,0) JIT fillers: scores of pair g read K(0, tb) for
            # tb <= (2g+1)//tpb; the (2-behind) A@V flush of pair g reads
            # V(2g), V(2g+1) — give those deadline (0,0,g) to be safe.
            for tb in range(1, TB):
                fillers.append(
                    ((0, 0, max(0, tb * tpb // 2 - 1)), lambda tb=tb: emit_k(0, tb))
                )
            for p in range(1, NPAIR):
                fillers.append(((0, 0, p), lambda t=2 * p: emit_v(t, 0)))
                fillers.append(((0, 0, p), lambda t=2 * p + 1: emit_v(t, 0)))
            fillers = deque(sorted(fillers, key=lambda f: f[0]))

            def enqueue(hp, sb):
                """Work that should ride under block (hp, sb)'s exp stream."""
                add = []
                if hp > 0 and sb == 0 and TT > 2:
                    # second half of this hp's V rides under (hp, 0) itself
                    for t in range(TT // 2, TT):
                        add.append(
                            ((hp, 0, t // 2), lambda t=t, hp=hp: emit_v(t, hp))
                        )
                if sb == SB - 1 and hp + 1 < HPAIRS:
                    nhp = hp + 1
                    for tb in range(TB):
                        dl = (nhp, 0, max(0, tb * tpb // 2 - 1))
                        add.append((dl, lambda tb=tb, nhp=nhp: emit_k(nhp, tb)))
                    for rb in range(QB):
                        add.append(((nhp, 0, 0), lambda rb=rb, nhp=nhp: emit_q(nhp, rb)))
                    vhi = TT // 2 if TT > 2 else TT
                    for t in range(vhi):
                        add.append(
                            ((nhp, 0, t // 2), lambda t=t, nhp=nhp: emit_v(t, nhp))
                        )
                if hp == HPAIRS - 1 and sb == SB - 1 and SB > 1:
                    # stream earlier s-blocks' output tiles under the last
                    # attention block (deadline: end of everything)
                    for st in range((SB - 1) * (ST // SB)):
                        def of(st=st):
                            emit_out(st)
                            emitted_out.add(st)
                        add.append(((HPAIRS, 0, 0), of))
                for item in sorted(add, key=lambda f: f[0]):
                    fillers.append(item)

            for hp in range(HPAIRS):
                for sb in range(SB):
                    enqueue(hp, sb)
                    pump = max(1, min(3, -(-len(fillers) // max(1, NPAIR))))
                    attention(hp, sb, pump,
                              last_block=(hp == HPAIRS - 1 and sb == SB - 1))

            while fillers:
                fillers.popleft()[1]()
            for st in range(ST):
                if st not in emitted_out:
                    emit_out(st)

    return nc


_CACHED = {}


def _get_nc(seq=S, rows=R, exp_group=2, sblk=512):
    key = (seq, rows, exp_group, sblk)
    if key not in _CACHED:
        nc = bacc.Bacc("TRN2", target_bir_lowering=False, debug=False,
                       num_devices=N_CORES)
        build_mha(nc, seq=seq, rows=rows, exp_group=exp_group, sblk=sblk)
        nc.compile()
        _CACHED[key] = nc
    return _CACHED[key]


def make_in_maps(inputs):
    """Shard full inputs into per-core input dicts."""
    bf = ml_dtypes.bfloat16
    x = np.asarray(inputs["x"], np.float32)
    shared = {
        "wq_bf": np.asarray(inputs["wq"], bf),
        "wk_bf": np.asarray(inputs["wk"], bf),
        "wv_bf": np.asarray(inputs["wv"], bf),
        "wo_bf": np.asarray(inputs["wo"], bf),
        "bq": np.asarray(inputs["bq"], np.float32),
        "bk": np.asarray(inputs["bk"], np.float32),
        "bv": np.asarray(inputs["bv"], np.float32),
        "bo": np.asarray(inputs["bo"], np.float32),
        "ln_g": np.asarray(inputs["ln_g"], np.float32),
        "ln_b": np.asarray(inputs["ln_b"], np.float32),
    }
    x_bf_all = [np.ascontiguousarray(x[b].astype(bf)) for b in range(B)]
    in_maps = []
    for c in range(N_CORES):
        b, shard = divmod(c, SEQ_SHARDS)
        r0 = shard * R
        m = dict(shared)
        m["x_bf"] = x_bf_all[b]
        m["xo_bf"] = np.ascontiguousarray(x_bf_all[b][r0 : r0 + R])
        m["xo_f32"] = np.ascontiguousarray(x[b, r0 : r0 + R])
        in_maps.append(m)
    return in_maps


def kernel(**inputs):
    from concourse import bass_utils

    nc = _get_nc()
    in_maps = make_in_maps(inputs)
    res = bass_utils.run_bass_kernel_spmd(nc, in_maps, core_ids=list(range(N_CORES)))
    out = np.empty((B, S, E), np.float32)
    for c in range(N_CORES):
        b, shard = divmod(c, SEQ_SHARDS)
        out[b, shard * R : (shard + 1) * R] = res.results[c]["y"]
    return out
